# revision 2
# baseline (speedup 1.0000x reference)
"""PointNet++ feature propagation on 8 NeuronCores (batch-parallel).

Per core (one batch): exact-f32-class 3-NN via split-fp32r distance matmul
(PE) + top-8 scan (DVE max/max_index on PSUM), inverse-distance weights,
interpolation as indirect-DMA gather of W0-projected features (fp16) +
diagonal-matmul weighting accumulated straight into the first conv's PSUM,
training-mode BatchNorm with cross-core AllReduce of stats, fused
scale+bias+ReLU applies on ACT, second conv, second BN+ReLU.
"""
import numpy as np

import concourse.bass as bass
import concourse.mybir as mybir
import concourse.tile as tile_mod
from concourse import tile
from concourse.bass_utils import run_bass_kernel_spmd
from concourse.vector_clock import ScopedClock
from concourse.bass import _add_dep_helper

dt = mybir.dt

B, N, M, C, CP = 8, 8192, 2048, 256, 512
O0, O1 = 256, 128
NT = N // 128           # 64 query tiles
MC = M // 512           # 4 distance matmul chunks
GRP = 8                 # tiles per small-op batch
BN_EPS = 1e-5

# ---------------------------------------------------------------------------
# workarounds: this walrus build accepts at most ONE sync wait per instruction
MAX_WAITS = 1
_wsplit_ctr = [0]


def _patched_drain_and_barrier(self, tick_clock, wait_clock):
    nc = self.nc
    drain_inst = nc.sync.drain()
    wait_clock.add_sem_waits(
        drain_inst.ins, ScopedClock({None: tick_clock.global_clock})
    )
    si = drain_inst.ins.sync_info
    if si is not None and si.on_wait is not None and len(si.on_wait) > MAX_WAITS:
        waits = list(si.on_wait)
        si.on_wait = waits[:MAX_WAITS]
        for _ in range(MAX_WAITS, len(waits), MAX_WAITS):
            extra = nc.sync.drain()
            if extra.ins.sync_info is None:
                extra.ins.sync_info = mybir.SyncInfo(on_wait=[], on_update=[])
        bb = nc.cur_bb.bb if hasattr(nc.cur_bb, "bb") else nc.cur_bb
        seen = False
        idx = MAX_WAITS
        for inst in bb.instructions:
            if inst is drain_inst.ins or inst.name == drain_inst.ins.name:
                seen = True
                continue
            if seen and inst.opcode == "Drain" and idx < len(waits):
                inst.sync_info.on_wait = waits[idx:idx + MAX_WAITS]
                idx += MAX_WAITS
    nc.all_engine_barrier()
    popped = nc._tile_sem_poison_stack.pop()
    assert popped is self._sem_poison
    nc.clear_and_free_semaphores(list(self.sems.allocated().values()))
    nc.all_engine_barrier()


tile_mod.TileContext._drain_and_barrier = _patched_drain_and_barrier


def _split_multi_waits(nc):
    for f in nc.m.functions:
        for bb in f.blocks:
            new_insts = []
            changed = False
            for inst in bb.instructions:
                si = inst.sync_info
                waits = list(si.on_wait) if (si is not None and si.on_wait) else []
                if len(waits) > MAX_WAITS and inst.engine != mybir.EngineType.SP:
                    changed = True
                    extra, keep = waits[:-MAX_WAITS], waits[-MAX_WAITS:]
                    for j in range(0, len(extra), MAX_WAITS):
                        _wsplit_ctr[0] += 1
                        nop = mybir.InstNoOp(
                            name=f"WSPLIT-{_wsplit_ctr[0]}", ins=[], outs=[])
                        nop.engine = inst.engine
                        nop.sync_info = mybir.SyncInfo(
                            on_wait=extra[j:j + MAX_WAITS], on_update=[])
                        new_insts.append(nop)
                    si.on_wait = keep
                new_insts.append(inst)
            if changed:
                bb.instructions.clear()
                for i in new_insts:
                    bb.add_instruction(i)


# ---------------------------------------------------------------------------
def _build_nc():
    nc = bass.Bass("TRN2", target_bir_lowering=False, debug=False, num_devices=8)

    d_lhs = nc.dram_tensor("lhs16", [16, N], dt.float32, kind="ExternalInput")
    d_rhs = nc.dram_tensor("rhs16", [16, M], dt.float32, kind="ExternalInput")
    d_feat = nc.dram_tensor("feat", [C, N], dt.float32, kind="ExternalInput")
    d_fp = nc.dram_tensor("fp", [CP, M], dt.float32, kind="ExternalInput")
    d_w0pt = nc.dram_tensor("w0pt", [CP, O0], dt.float32, kind="ExternalInput")
    d_w0ft = nc.dram_tensor("w0ft", [C, O0], dt.float32, kind="ExternalInput")
    d_w1t = nc.dram_tensor("w1t", [O0, O1], dt.float32, kind="ExternalInput")
    d_eye = nc.dram_tensor("eye", [128, 128], dt.float16, kind="ExternalInput")
    d_g0 = nc.dram_tensor("g0", [128, 2], dt.float32, kind="ExternalInput")
    d_be0 = nc.dram_tensor("be0", [128, 2], dt.float32, kind="ExternalInput")
    d_g1 = nc.dram_tensor("g1", [128, 1], dt.float32, kind="ExternalInput")
    d_be1 = nc.dram_tensor("be1", [128, 1], dt.float32, kind="ExternalInput")
    d_out = nc.dram_tensor("out", [O1, N], dt.float32, kind="ExternalOutput")
    import os
    KDEBUG = os.environ.get("KDEBUG", "0") == "1"
    KSKIP = os.environ.get("KSKIP", "")
    if KDEBUG:
        d_dbg_v = nc.dram_tensor("dbg_vals", [128, NT * 8], dt.float32, kind="ExternalOutput")
        d_dbg_i = nc.dram_tensor("dbg_idxs", [128, NT * 8], dt.uint32, kind="ExternalOutput")
        d_dbg_y0 = nc.dram_tensor("dbg_y0", [2, 128, N], dt.float32, kind="ExternalOutput")
        d_dbg_ar0 = nc.dram_tensor("dbg_ar0", [128, 4], dt.float32, kind="ExternalOutput")
        d_dbg_w = nc.dram_tensor("dbg_w", [128, NT * 3], dt.float32, kind="ExternalOutput")
        d_dbg_zt = nc.dram_tensor("dbg_zt", [M, O0], dt.float16, kind="ExternalOutput")
        d_dbg_gt = nc.dram_tensor("dbg_gt", [128, 3, O0], dt.float16, kind="ExternalOutput")
        d_dbg_dj = nc.dram_tensor("dbg_dj", [128, 128], dt.float16, kind="ExternalOutput")

    d_zt = nc.dram_tensor("ztab", [M, O0], dt.float16)
    d_ar0i = nc.dram_tensor("ar0i", [128, 4], dt.float32)
    d_ar0o = nc.dram_tensor("ar0o", [128, 4], dt.float32, addr_space="Shared")
    d_ar1i = nc.dram_tensor("ar1i", [128, 2], dt.float32)
    d_ar1o = nc.dram_tensor("ar1o", [128, 2], dt.float32, addr_space="Shared")

    RG = [[0, 1, 2, 3, 4, 5, 6, 7]]
    AF = mybir.ActivationFunctionType

    with tile.TileContext(nc) as tc:
        with tc.tile_pool(name="persist", bufs=1) as pp, \
             tc.tile_pool(name="spool", bufs=1, space="PSUM") as sp, \
             tc.tile_pool(name="ypool", bufs=2, space="PSUM") as yp:

            # ---------- phase 0: load + preprocess ----------
            rhs_r = pp.tile([16, M], dt.float32r)
            lhs_r = pp.tile([16, N], dt.float32r)
            with tc.tile_pool(name="ph0", bufs=1) as tp:
                t_rhs = tp.tile([16, M], dt.float32, tag="ldr")
                nc.gpsimd.dma_start(t_rhs[:], d_rhs[:])
                nc.vector.tensor_copy(rhs_r[:], t_rhs[:])

                t_lhs = tp.tile([16, N], dt.float32, tag="ldl")
                nc.gpsimd.dma_start(t_lhs[:], d_lhs[:])
                nc.vector.tensor_copy(lhs_r[:], t_lhs[:])

            t_eye = pp.tile([128, 128], dt.float16)
            nc.gpsimd.dma_start(t_eye[:], d_eye[:])
            t_g0 = pp.tile([128, 2], dt.float32)
            nc.gpsimd.dma_start(t_g0[:], d_g0[:])
            t_be0 = pp.tile([128, 2], dt.float32)
            nc.gpsimd.dma_start(t_be0[:], d_be0[:])
            t_g1 = pp.tile([128, 1], dt.float32)
            nc.gpsimd.dma_start(t_g1[:], d_g1[:])
            t_be1 = pp.tile([128, 1], dt.float32)
            nc.gpsimd.dma_start(t_be1[:], d_be1[:])

            # Z table: ZT[m, o] = sum_c fp[c, m] * W0pT[c, o], stored fp16 in DRAM
            zt_store_insts = []
            with tc.tile_pool(name="zbuild", bufs=1) as zb:
                w0p_r = []
                for kc in range(4):
                    t_w = zb.tile([128, O0], dt.float32, tag="w0pl")
                    nc.gpsimd.dma_start(t_w[:], d_w0pt[128 * kc:128 * (kc + 1), :])
                    w_r = zb.tile([128, O0], dt.float32r, tag=f"w0pr{kc}")
                    nc.vector.tensor_copy(w_r[:], t_w[:])
                    w0p_r.append(w_r)
                fp_r = []
                for kc in range(4):
                    t_f = zb.tile([128, M], dt.float32, tag="fpl")
                    nc.gpsimd.dma_start(t_f[:], d_fp[128 * kc:128 * (kc + 1), :])
                    f_r = zb.tile([128, M], dt.float32r, tag=f"fpr{kc}")
                    nc.vector.tensor_copy(f_r[:], t_f[:])
                    fp_r.append(f_r)
                for mt in range(M // 128):
                    zps = yp.tile([128, O0], dt.float32, tag="ypsum")
                    for kc in range(4):
                        nc.tensor.matmul(
                            zps[:], fp_r[kc][:, 128 * mt:128 * (mt + 1)],
                            w0p_r[kc][:], start=(kc == 0), stop=(kc == 3))
                    z_sb = zb.tile([128, O0], dt.float16, tag="zsb")
                    nc.scalar.copy(z_sb[:], zps[:])
                    st = nc.gpsimd.dma_start(
                        d_zt[128 * mt:128 * (mt + 1), :], z_sb[:])
                    zt_store_insts.append(st.ins)

            # token marking ZT completely stored (gathers must wait for it)
            zt_token = pp.tile([1, 1], dt.float32)
            tok = nc.gpsimd.memset(zt_token[:], 0.0)
            for st in zt_store_insts:
                _add_dep_helper(tok.ins, st, sync=True, reason="zt ready")

            w0f_sb = []
            for kc in range(2):
                t_w0f = pp.tile([128, O0], dt.float32, tag=f"w0f{kc}", name=f"w0f{kc}")
                nc.gpsimd.dma_start(t_w0f[:], d_w0ft[128 * kc:128 * (kc + 1), :])
                w0f_sb.append(t_w0f)
            w1t_sb = []
            for kc in range(2):
                t_w = pp.tile([128, O1], dt.float32, tag=f"w1t{kc}")
                nc.gpsimd.dma_start(t_w[:], d_w1t[128 * kc:128 * (kc + 1), :])
                w1t_sb.append(t_w)

            if KDEBUG:
                wall = pp.tile([128, NT * 3], dt.float32, name="wall")
            # persistent buffers
            vals = pp.tile([128, NT * 8], dt.float32)    # top-8 s values per tile
            idxs = pp.tile([128, NT * 8], dt.uint32)     # top-8 indices per tile
            y0raw = []
            for oc in range(2):
                y0r = pp.tile([128, N], dt.float32, tag=f"y0raw{oc}", name=f"y0raw{oc}")
                y0raw.append(y0r)
            s0sl = pp.tile([128, 2 * NT], dt.float32)    # layer0 sums (oc-major)
            q0sl = pp.tile([128, 2 * NT], dt.float32)
            s1sl = pp.tile([128, NT], dt.float32)
            q1sl = pp.tile([128, NT], dt.float32)

            # ---------- phase 1 ----------
            with tc.tile_pool(name="p1", bufs=2) as p1:
                for t in range(NT):
                    sl = slice(128 * t, 128 * (t + 1))
                    sps = sp.tile([128, M], dt.float32, tag="smat")
                    for mc in range(MC):
                        nc.tensor.matmul(
                            sps[:, 512 * mc:512 * (mc + 1)], lhs_r[:, sl],
                            rhs_r[:, 512 * mc:512 * (mc + 1)],
                            start=True, stop=True)
                    vsl = vals[:, 8 * t:8 * (t + 1)]
                    isl = idxs[:, 8 * t:8 * (t + 1)]
                    nc.vector.max(vsl, sps[:])
                    nc.vector.max_index(isl, vsl, sps[:])

                    g, r = divmod(t, GRP)
                    if r == GRP - 1:
                        # weights for tiles [g*GRP, (g+1)*GRP): w = inv/sum(inv)
                        gs = slice(8 * GRP * g, 8 * GRP * (g + 1))
                        v_view = vals[:, gs].rearrange("p (t e) -> p t e", e=8)[:, :, 0:3]
                        d2 = p1.tile([128, GRP, 3], dt.float32, tag="d2")
                        nc.gpsimd.tensor_scalar(
                            d2[:], v_view, -1.0, 0.0,
                            op0=mybir.AluOpType.mult, op1=mybir.AluOpType.max)
                        nc.gpsimd.tensor_scalar(
                            d2[:], d2[:], 1e-8, None, op0=mybir.AluOpType.add)
                        inv = p1.tile([128, GRP, 3], dt.float32, tag="inv")
                        nc.vector.reciprocal(inv[:], d2[:])
                        ws = p1.tile([128, GRP], dt.float32, tag="ws")
                        nc.vector.tensor_reduce(
                            ws[:], inv[:], op=mybir.AluOpType.add,
                            axis=mybir.AxisListType.X)
                        wsi = p1.tile([128, GRP], dt.float32, tag="wsi")
                        nc.vector.reciprocal(wsi[:], ws[:])
                        wg = p1.tile([128, GRP, 3], dt.float32, tag=f"wg{g}")
                        nc.vector.tensor_tensor(
                            wg[:], inv[:],
                            wsi[:].rearrange("p (t o) -> p t o", o=1).broadcast_to([128, GRP, 3]),
                            op=mybir.AluOpType.mult)

                        if KDEBUG:
                            nc.gpsimd.tensor_copy(
                                wall[:, GRP * 3 * g:GRP * 3 * (g + 1)],
                                wg[:].rearrange("p t e -> p (t e)"))
                        for tt in range(GRP * g, GRP * (g + 1)):
                            tsl = slice(128 * tt, 128 * (tt + 1))
                            gt = p1.tile([128, 3, O0], dt.float16, tag="gt")
                            for j in range(3):
                                gi = nc.gpsimd.indirect_dma_start(
                                    out=gt[:, j, :], out_offset=None, in_=d_zt[:],
                                    in_offset=bass.IndirectOffsetOnAxis(
                                        ap=idxs[:, 8 * tt + j:8 * tt + j + 1], axis=0),
                                )
                                _add_dep_helper(gi.ins, tok.ins, sync=True,
                                                reason="gather after zt")
                            ftile = []
                            for kc in range(2):
                                f_t = p1.tile([128, 128], dt.float32,
                                              tag=f"ft{kc}", name=f"ft{kc}_{tt}")
                                nc.gpsimd.dma_start(
                                    f_t[:], d_feat[128 * kc:128 * (kc + 1), tsl])
                                ftile.append(f_t)
                            if KDEBUG and tt == 0:
                                nc.gpsimd.dma_start(d_dbg_gt[:], gt[:])
                            y0ps = yp.tile([128, 2 * 128], dt.float32, tag="ypsum")
                            djs = []
                            for j in range(3):
                                dj = p1.tile([128, 128], dt.float16,
                                             tag=f"diag{j}", name=f"dj{j}_{tt}")
                                nc.scalar.activation(
                                    dj[:], t_eye[:], AF.Copy,
                                    scale=wg[:, tt - GRP * g, j:j + 1])
                                djs.append(dj)
                            for oc in range(2):
                                osl = slice(128 * oc, 128 * (oc + 1))
                                for kc in range(2):
                                    nc.tensor.matmul(
                                        y0ps[:, osl], w0f_sb[kc][:, osl],
                                        ftile[kc][:],
                                        start=(kc == 0), stop=False)
                                for j in range(3):
                                    nc.tensor.matmul(
                                        y0ps[:, osl], gt[:, j, osl], djs[j][:],
                                        start=False, stop=(j == 2))
                            for oc in range(2):
                                osl = slice(128 * oc, 128 * (oc + 1))
                                nc.scalar.activation(
                                    y0raw[oc][:, tsl], y0ps[:, osl], AF.Copy,
                                    accum_out=s0sl[:, NT * oc + tt:NT * oc + tt + 1])
                                scrap = p1.tile([128, 128], dt.float32, tag="scrap")
                                nc.scalar.activation(
                                    scrap[:], y0ps[:, osl], AF.Square,
                                    accum_out=q0sl[:, NT * oc + tt:NT * oc + tt + 1])

                # ---------- allreduce layer-0 stats ----------
                ar0 = pp.tile([128, 4], dt.float32)
                nc.vector.tensor_reduce(
                    ar0[:, 0:1], s0sl[:, 0:NT], op=mybir.AluOpType.add,
                    axis=mybir.AxisListType.X)
                nc.vector.tensor_reduce(
                    ar0[:, 1:2], s0sl[:, NT:2 * NT], op=mybir.AluOpType.add,
                    axis=mybir.AxisListType.X)
                nc.vector.tensor_reduce(
                    ar0[:, 2:3], q0sl[:, 0:NT], op=mybir.AluOpType.add,
                    axis=mybir.AxisListType.X)
                nc.vector.tensor_reduce(
                    ar0[:, 3:4], q0sl[:, NT:2 * NT], op=mybir.AluOpType.add,
                    axis=mybir.AxisListType.X)
                st0 = nc.gpsimd.dma_start(d_ar0i[:], ar0[:])
                cc0 = nc.gpsimd.collective_compute(
                    "AllReduce", mybir.AluOpType.add, replica_groups=RG,
                    ins=[d_ar0i[:]], outs=[d_ar0o[:]])
                _add_dep_helper(cc0.ins, st0.ins, sync=True, reason="ar0 in")
                ar0r = pp.tile([128, 4], dt.float32)
                ld0 = nc.gpsimd.dma_start(ar0r[:], d_ar0o[:])
                _add_dep_helper(ld0.ins, cc0.ins, sync=True, reason="ar0 out")

                # finalize a0 = g0*rsqrt(var+eps), c0 = be0 - mean*a0
                cnt = float(B * N)
                mean0 = pp.tile([128, 2], dt.float32)
                nc.vector.tensor_scalar_mul(mean0[:], ar0r[:, 0:2], 1.0 / cnt)
                var0 = pp.tile([128, 2], dt.float32)
                nc.vector.tensor_scalar_mul(var0[:], ar0r[:, 2:4], 1.0 / cnt)
                msq0 = pp.tile([128, 2], dt.float32)
                nc.vector.tensor_tensor(
                    msq0[:], mean0[:], mean0[:], op=mybir.AluOpType.mult)
                nc.vector.tensor_tensor(
                    var0[:], var0[:], msq0[:], op=mybir.AluOpType.subtract)
                nc.vector.tensor_scalar_add(var0[:], var0[:], BN_EPS)
                sd0 = pp.tile([128, 2], dt.float32)
                nc.scalar.activation(sd0[:], var0[:], AF.Sqrt)
                isd0 = pp.tile([128, 2], dt.float32)
                nc.vector.reciprocal(isd0[:], sd0[:])
                a0 = pp.tile([128, 2], dt.float32)
                nc.vector.tensor_tensor(a0[:], t_g0[:], isd0[:],
                                        op=mybir.AluOpType.mult)
                c0 = pp.tile([128, 2], dt.float32)
                nc.vector.tensor_tensor(c0[:], mean0[:], a0[:],
                                        op=mybir.AluOpType.mult)
                nc.vector.tensor_tensor(c0[:], t_be0[:], c0[:],
                                        op=mybir.AluOpType.subtract)

            if KDEBUG:
                nc.gpsimd.dma_start(d_dbg_v[:], vals[:])
                nc.gpsimd.dma_start(d_dbg_i[:], idxs[:])
                for oc in range(2):
                    nc.gpsimd.dma_start(d_dbg_y0[oc], y0raw[oc][:])
                nc.gpsimd.dma_start(d_dbg_ar0[:], ar0r[:])
                nc.gpsimd.dma_start(d_dbg_w[:], wall[:])
                nc.gpsimd.dma_start(d_dbg_zt[:], d_zt[:])
            # ---------- phase 2: h = relu(a0*y0+c0); y1 = W1 @ h ----------
            y1raw = pp.tile([128, N], dt.float32)
            with tc.tile_pool(name="p2", bufs=3) as p2:
                for t in range(NT):
                    tsl = slice(128 * t, 128 * (t + 1))
                    h = []
                    for oc in range(2):
                        h_t = p2.tile([128, 128], dt.float32, tag=f"h{oc}", name=f"h{oc}_{t}")
                        h.append(h_t)
                    for oc in range(2):
                        nc.scalar.activation(
                            h[oc][:], y0raw[oc][:, tsl], AF.Relu,
                            scale=a0[:, oc:oc + 1], bias=c0[:, oc:oc + 1])
                    y1ps = yp.tile([128, 2 * 128], dt.float32, tag="ypsum")
                    for kc in range(2):
                        nc.tensor.matmul(y1ps[:, 0:128], w1t_sb[kc][:], h[kc][:],
                                         start=(kc == 0), stop=(kc == 1))
                    nc.scalar.activation(
                        y1raw[:, tsl], y1ps[:, 0:128], AF.Copy,
                        accum_out=s1sl[:, t:t + 1])
                    scrap2 = p2.tile([128, 128], dt.float32, tag="scrap2")
                    nc.scalar.activation(
                        scrap2[:], y1ps[:, 0:128], AF.Square,
                        accum_out=q1sl[:, t:t + 1])

                ar1 = pp.tile([128, 2], dt.float32)
                nc.vector.tensor_reduce(
                    ar1[:, 0:1], s1sl[:], op=mybir.AluOpType.add,
                    axis=mybir.AxisListType.X)
                nc.vector.tensor_reduce(
                    ar1[:, 1:2], q1sl[:], op=mybir.AluOpType.add,
                    axis=mybir.AxisListType.X)
                st1 = nc.gpsimd.dma_start(d_ar1i[:], ar1[:])
                cc1 = nc.gpsimd.collective_compute(
                    "AllReduce", mybir.AluOpType.add, replica_groups=RG,
                    ins=[d_ar1i[:]], outs=[d_ar1o[:]])
                _add_dep_helper(cc1.ins, st1.ins, sync=True, reason="ar1 in")
                ar1r = pp.tile([128, 2], dt.float32)
                ld1 = nc.gpsimd.dma_start(ar1r[:], d_ar1o[:])
                _add_dep_helper(ld1.ins, cc1.ins, sync=True, reason="ar1 out")

                cnt = float(B * N)
                mean1 = pp.tile([128, 1], dt.float32)
                nc.vector.tensor_scalar_mul(mean1[:], ar1r[:, 0:1], 1.0 / cnt)
                var1 = pp.tile([128, 1], dt.float32)
                nc.vector.tensor_scalar_mul(var1[:], ar1r[:, 1:2], 1.0 / cnt)
                msq1 = pp.tile([128, 1], dt.float32)
                nc.vector.tensor_tensor(
                    msq1[:], mean1[:], mean1[:], op=mybir.AluOpType.mult)
                nc.vector.tensor_tensor(
                    var1[:], var1[:], msq1[:], op=mybir.AluOpType.subtract)
                nc.vector.tensor_scalar_add(var1[:], var1[:], BN_EPS)
                sd1 = pp.tile([128, 1], dt.float32)
                nc.scalar.activation(sd1[:], var1[:], AF.Sqrt)
                isd1 = pp.tile([128, 1], dt.float32)
                nc.vector.reciprocal(isd1[:], sd1[:])
                a1 = pp.tile([128, 1], dt.float32)
                nc.vector.tensor_tensor(a1[:], t_g1[:], isd1[:],
                                        op=mybir.AluOpType.mult)
                c1 = pp.tile([128, 1], dt.float32)
                nc.vector.tensor_tensor(c1[:], mean1[:], a1[:],
                                        op=mybir.AluOpType.mult)
                nc.vector.tensor_tensor(c1[:], t_be1[:], c1[:],
                                        op=mybir.AluOpType.subtract)

            # ---------- phase 3: out = relu(a1*y1+c1) ----------
            with tc.tile_pool(name="p3", bufs=3) as p3:
                for t in range(NT):
                    tsl = slice(128 * t, 128 * (t + 1))
                    o = p3.tile([128, 128], dt.float32, tag="o")
                    nc.scalar.activation(o[:], y1raw[:, tsl], AF.Relu,
                                         scale=a1[:], bias=c1[:])
                    nc.gpsimd.dma_start(d_out[:, tsl], o[:])

    _split_multi_waits(nc)
    return nc


_NC_CACHE = []


def _get_nc():
    if not _NC_CACHE:
        _NC_CACHE.append(_build_nc())
    return _NC_CACHE[0]


def _split12(v):
    """x = a + b with a = top-12-bit part (both fp32r-exact)."""
    a = np.floor(v * 4096.0) / np.float32(4096.0)
    a = a.astype(np.float32)
    b = (v - a).astype(np.float32)
    return a, b


def _split_f16(v):
    hi = np.float16(v).astype(np.float32)
    lo = (v - hi).astype(np.float32)
    return hi, lo


def kernel(xyz, xyz_prev, features, features_prev,
           W0, b0, g0, be0, W1, b1, g1, be1):
    xyz = np.asarray(xyz, np.float32)
    xyz_prev = np.asarray(xyz_prev, np.float32)
    features = np.ascontiguousarray(np.asarray(features, np.float32))
    features_prev = np.ascontiguousarray(np.asarray(features_prev, np.float32))
    W0 = np.asarray(W0, np.float32)
    W1 = np.asarray(W1, np.float32)

    w0pt = np.ascontiguousarray(W0[:, :CP].T)
    w0ft = np.ascontiguousarray(W0[:, CP:].T)
    w1t = np.ascontiguousarray(W1.T)
    eye = np.eye(128, dtype=np.float16)
    g0d = np.ascontiguousarray(np.asarray(g0, np.float32).reshape(2, 128).T)
    be0d = np.ascontiguousarray(np.asarray(be0, np.float32).reshape(2, 128).T)
    g1d = np.asarray(g1, np.float32).reshape(1, 128).T.copy()
    be1d = np.asarray(be1, np.float32).reshape(1, 128).T.copy()

    in_maps = []
    for bb_ in range(B):
        x = xyz[bb_]                       # [N, 3]
        p = xyz_prev[bb_]                  # [M, 3]
        xa, xb = _split12(x)
        pa, pb = _split12(p)
        nx2 = (x * x).sum(-1, dtype=np.float32)
        np2 = (p * p).sum(-1, dtype=np.float32)
        nxh, nxl = _split_f16(nx2)
        nph, npl = _split_f16(np2)

        lhs16 = np.empty((16, N), np.float32)
        rhs16 = np.empty((16, M), np.float32)
        for c in range(3):
            lhs16[c] = 2.0 * xa[:, c]
            lhs16[3 + c] = 2.0 * xa[:, c]
            lhs16[6 + c] = 2.0 * xb[:, c]
            lhs16[9 + c] = 2.0 * xb[:, c]
            rhs16[c] = pa[:, c]
            rhs16[3 + c] = pb[:, c]
            rhs16[6 + c] = pa[:, c]
            rhs16[9 + c] = pb[:, c]
        lhs16[12] = 1.0
        lhs16[13] = 1.0
        lhs16[14] = -nxh
        lhs16[15] = -nxl
        rhs16[12] = -nph
        rhs16[13] = -npl
        rhs16[14] = 1.0
        rhs16[15] = 1.0

        in_maps.append({
            "lhs16": lhs16, "rhs16": rhs16,
            "feat": features[bb_], "fp": features_prev[bb_],
            "w0pt": w0pt, "w0ft": w0ft, "w1t": w1t, "eye": eye,
            "g0": g0d, "be0": be0d, "g1": g1d, "be1": be1d,
        })

    nc = _get_nc()
    import os as _os
    _kw = {}
    if _os.environ.get("KTRACE", "0") == "1":
        _tdir = "/tmp/ktrace"
        _os.makedirs(_tdir, exist_ok=True)
        _kw = dict(trace=True, tmpdir=_tdir)
    res = run_bass_kernel_spmd(nc, in_maps, list(range(B)), **_kw)
    global LAST_HW_NS, LAST_RES
    if getattr(res, "exec_time_ns", None):
        LAST_HW_NS = res.exec_time_ns
    LAST_RES = res
    out = np.stack([res.results[i]["out"] for i in range(B)], axis=0)
    return out


LAST_HW_NS = None
LAST_RES = None

